# revision 28
# baseline (speedup 1.0000x reference)
"""Trainium2 Bass kernel for a 2-layer dense-adjacency GCN.

    h      = relu(adj @ (x @ W1))          # [N, H]
    logits = adj @ (h @ W2) + b2           # [N, C]
    out    = log_softmax(logits, axis=-1)

Shapes: N=16384, F=512, H=128, C=8, fp32 inputs, 8 NeuronCores.

Sharding: 1-D row partition of adj/x (2048 rows per core); W1/W2/b2
replicated. Per-core xW1 and hW2 shards are all-gathered (DRAM-bounce
collectives) so each core can run its local adj_block products.

Orientation: the PE computes out = lhsT.T @ rhs with the contraction on
partitions for both operands, so adj tiles must be transposed on chip.
The transposes are regular bf16 matmuls against the identity (lhsT.T @ I;
same cycles as transpose-mode but HAM-countable). adj is cast to bf16
during the DMA load (fp32 matmul is 4x slower on the PE); transposed
tiles are scaled by 2^16 (adj values are O(1/N)) and written as fp8
e4m3. Both GCN layers then run fp8 DoubleRow matmuls (two interleaved
k-planes per instruction -> half the PE cycles): xW1 is carried in fp8
(values O(1)), hW2 in fp8 scaled by 2^12, and the combined scales are
divided out of the accumulators. Pass 2 re-reads adj from the fp8 adj^T
DRAM scratch (quarter the bytes of the fp32 original). Numerics land at
~7e-5 max abs error vs the fp32 reference: adj/xW1 quantization noise
averages out over the 16384-term positive-weighted contraction.

Scheduling notes (profiled): per k-chunk, four 128x128 PE transposes
write one [128, 512] PSUM tile and a single wide DVE/ACT op does the
scale+cast; the first slabs' transposes are emitted without their
matmuls so the PE has work while the xW1 all-gather is in flight
(engine streams are FIFO -- a waiting matmul at the head blocks
everything behind it). Collectives live on the gpsimd queue, so all
non-cast DMA (scratch store/load, outputs) goes through HWDGE (sync)
to keep that queue from blocking adj loads. Gathered xW1/hW2 are
distributed chunk-major via a contiguous load + PE transposes; a
direct strided DMA costs tens of microseconds in descriptor overhead.
"""

import os

import numpy as np

import concourse.mybir as mybir
import concourse.tile as tile
from concourse import bacc
from concourse.bass_utils import run_bass_kernel_spmd
from concourse.masks import make_identity

N = 16384
F = 512
H = 128
C = 8
NCORES = 8
R = N // NCORES          # rows per core (2048)
NT = 4                   # row tiles per core
RT = R // NT             # rows per row tile (512)
SUB = RT // 128          # 128-row sub-blocks per row tile (4)
KS = 8                   # column slabs per pass
KW = N // KS             # columns per slab (2048)
KC = KW // 128           # 128-col chunks per slab (16)
K2 = N // 256            # DoubleRow k-pair count (64)
PRE = 8                  # slabs pre-transposed ahead of the xW1 gather
NCACHE = 6               # (nt=3, ks>=KS-NCACHE) adjT tiles kept in SBUF
                         # across the pass boundary (skip store+reload)
SCALE = 65536.0          # adj values are O(1/N); keep fp8/bf16 in range
S2 = 4096.0              # hW2 values are O(1e-2); fp8 scale for layer 2
CP = 16                  # hW2 chunk stride pad (fp8 DoubleRow needs %16==0)

DT = mybir.dt
AF = mybir.ActivationFunctionType
DR = mybir.MatmulPerfMode.DoubleRow

_cached = None


def _build():
    nc = bacc.Bacc(
        "TRN2", target_bir_lowering=False, debug=False, num_devices=NCORES
    )

    x_loc = nc.dram_tensor("x_loc", [R, F], DT.float32, kind="ExternalInput")
    adj_loc = nc.dram_tensor("adj_loc", [R, N], DT.float32, kind="ExternalInput")
    w1_in = nc.dram_tensor("w1_in", [F, H], DT.float32, kind="ExternalInput")
    w2_in = nc.dram_tensor("w2_in", [H, C], DT.float32, kind="ExternalInput")
    b2_in = nc.dram_tensor("b2_in", [1, C], DT.float32, kind="ExternalInput")
    out_loc = nc.dram_tensor("out_loc", [R, C], DT.float32, kind="ExternalOutput")

    with tile.TileContext(nc) as tc:
        with (
            tc.tile_pool(name="consts", bufs=1) as consts,
            tc.tile_pool(name="slab", bufs=3) as slab_pool,
            tc.tile_pool(name="atp", bufs=12) as at_pool,
            tc.tile_pool(name="work", bufs=2) as work,
            tc.tile_pool(name="pp_tp", bufs=3, space="PSUM") as pp_tp,
            tc.tile_pool(name="pp_acc", bufs=2, space="PSUM") as pp_acc,
            tc.tile_pool(name="pp_small", bufs=2, space="PSUM") as pp_small,
            tc.tile_pool(name="dram", bufs=1, space="DRAM") as dram,
        ):
            # adj^T scratch for pass 2 (scaled by SCALE, fp8 e4m3).
            # Private spill: blocks stored in at-tile shape, contiguous per
            # partition -- a canonical [N, R] layout costs 4x store bandwidth
            # in 512-byte strided descriptors.
            adjt8 = dram.tile([NT, KS, 128, KC, RT], DT.float8e4)

            def load_slab(nt, ks):
                # split into per-subblock DMAs so transposes can start on
                # s=0 while s=1..3 are still in flight
                a_sb = slab_pool.tile([128, SUB, KW], DT.bfloat16, tag="a")
                for s in range(SUB):
                    nc.gpsimd.dma_start(
                        out=a_sb[:, s, :],
                        in_=adj_loc[
                            nt * RT + s * 128 : nt * RT + (s + 1) * 128,
                            ks * KW : (ks + 1) * KW,
                        ],
                    )
                return a_sb

            def transpose_slab(a_sb, nt, ks):
                # 4 PE transposes -> one [128, 512] PSUM tile -> one wide
                # scale+cast copy per k-chunk; fp8 output doubles as matmul
                # rhs and scratch-store source
                at_sb = at_pool.tile([128, KC, RT], DT.float8e4, tag="at")
                for kc in range(KC):
                    at_ps = pp_tp.tile([128, RT], DT.float32, tag="tp")
                    for s in range(SUB):
                        nc.tensor.matmul(
                            at_ps[:, s * 128 : (s + 1) * 128],
                            a_sb[:, s, kc * 128 : (kc + 1) * 128],
                            ident_bf[:],
                            start=True,
                            stop=True,
                        )
                    if kc % 3 != 2:  # DVE copy is ~1.6x faster than ACT
                        nc.vector.tensor_scalar_mul(at_sb[:, kc, :], at_ps[:], SCALE)
                    else:
                        nc.scalar.mul(at_sb[:, kc, :], at_ps[:], SCALE)
                if nt == NT - 1 and ks >= KS - NCACHE:
                    # cached across the pass boundary; no spill
                    return at_sb
                # ACT's HWDGE ring: stores must not serialize against the
                # SP-ring loads
                nc.scalar.dma_start(out=adjt8[nt, ks], in_=at_sb[:])
                return at_sb

            def mm_slab(ht_ps, at_sb, ks):
                # fp8 DoubleRow: two k-chunks per matmul
                for kd in range(KC // 2):
                    k2 = ks * (KC // 2) + kd
                    nc.tensor.matmul(
                        ht_ps[:],
                        xw1f_sb[:, 2 * k2 : 2 * k2 + 2, :],
                        at_sb[:, 2 * kd : 2 * kd + 2, :],
                        start=(k2 == 0),
                        stop=(k2 == K2 - 1),
                        perf_mode=DR,
                    )

            # ---------------- constants + stage-X input first -----------------
            ident_bf = consts.tile([128, 128], DT.bfloat16)
            make_identity(nc, ident_bf[:])
            ident_f32 = consts.tile([128, 128], DT.float32)
            make_identity(nc, ident_f32[:])

            # x_sb[p, s, f] = x_loc[s*128 + p, f]  (bf16, cast in DMA)
            # shares slab slots (same 16 KB/partition footprint)
            x_sb = slab_pool.tile([128, R // 128, F], DT.bfloat16, tag="a")
            nc.gpsimd.dma_start(
                out=x_sb[:], in_=x_loc[:, :].rearrange("(s p) f -> p s f", p=128)
            )
            w1_sb = consts.tile([128, F // 128, H], DT.bfloat16)
            nc.gpsimd.dma_start(
                out=w1_sb[:], in_=w1_in[:, :].rearrange("(c p) n -> p c n", p=128)
            )
            w2_sb = consts.tile([128, C], DT.bfloat16)
            nc.gpsimd.dma_start(out=w2_sb[:], in_=w2_in[:, :])
            b2_sb = consts.tile([C, 1], DT.float32)
            nc.sync.dma_start(out=b2_sb[:], in_=b2_in[:, :].rearrange("a b -> b a"))

            # ---------------- stage X: xW1 shard (fp8 out) ----------------
            xw1_sb = work.tile([128, R // 128, H], DT.float8e4, bufs=1)
            for s in range(R // 128):
                xt_ps = pp_tp.tile([128, F], DT.float32, tag="tp")
                for c in range(F // 128):
                    nc.tensor.matmul(
                        xt_ps[:, c * 128 : (c + 1) * 128],
                        x_sb[:, s, c * 128 : (c + 1) * 128],
                        ident_bf[:],
                        start=True,
                        stop=True,
                    )
                xt_sb = work.tile([128, F], DT.bfloat16, tag="xt")
                if s % 2 == 0:
                    nc.vector.tensor_copy(xt_sb[:], xt_ps[:])
                else:
                    nc.scalar.copy(xt_sb[:], xt_ps[:])
                xw1_ps = pp_small.tile([128, H], DT.float32, tag="small")
                for c in range(F // 128):
                    nc.tensor.matmul(
                        xw1_ps[:],
                        xt_sb[:, c * 128 : (c + 1) * 128],
                        w1_sb[:, c, :],
                        start=(c == 0),
                        stop=(c == F // 128 - 1),
                    )
                if s % 2 == 0:
                    nc.vector.tensor_copy(xw1_sb[:, s, :], xw1_ps[:])
                else:
                    nc.scalar.copy(xw1_sb[:, s, :], xw1_ps[:])

            xw1_bounce = dram.tile([R, H], DT.float8e4)
            nc.sync.dma_start(
                out=xw1_bounce[:, :].rearrange("(s p) n -> p s n", p=128),
                in_=xw1_sb[:],
            )

            # pre-transpose slabs; their matmuls are emitted after the gather
            pre_at = {}
            for ks in range(PRE):
                a_sb = load_slab(0, ks)
                pre_at[ks] = transpose_slab(a_sb, 0, ks)

            # ---------------- all-gather xW1 (fp8, 2 MiB) ----------------
            xw1_all = dram.tile([N, H], DT.float8e4, addr_space="Shared")
            nc.gpsimd.collective_compute(
                "AllGather",
                mybir.AluOpType.bypass,
                replica_groups=[list(range(NCORES))],
                ins=[xw1_bounce.opt()],
                outs=[xw1_all.opt()],
            )
            # chunk-major lhsT layout, loaded in 8-chunk blocks so the k=0
            # matmuls start as soon as the first block lands (an all-at-once
            # distribution costs ~40us of PE idle after the gather)
            # xw1f_sb[q, c, m] = xW1[c*128 + q, m]
            xw1f_sb = consts.tile([128, N // 128, H], DT.float8e4)
            for b in range(N // 128 // 8):
                nc.sync.dma_start(
                    out=xw1f_sb[:, b * 8 : (b + 1) * 8, :],
                    in_=xw1_all[b * 1024 : (b + 1) * 1024, :].rearrange(
                        "(c p) m -> p c m", p=128
                    ),
                )

            # ---------------- pass 1: hT = (adj_loc @ xW1)^T ----------------
            hw2p_sb = work.tile([128, R // 128, C], DT.bfloat16, bufs=1)
            cached = {}
            for nt in range(NT):
                ht_ps = pp_acc.tile([128, RT], DT.float32, tag="acc")
                for ks in range(KS):
                    if nt == 0 and ks < PRE:
                        at_sb = pre_at.pop(ks)
                    else:
                        a_sb = load_slab(nt, ks)
                        at_sb = transpose_slab(a_sb, nt, ks)
                    if nt == NT - 1 and ks >= KS - NCACHE:
                        cached[(nt, ks)] = at_sb
                    mm_slab(ht_ps, at_sb, ks)
                # hT tile for this nt: relu + descale, bf16
                ht_sb = work.tile([128, RT], DT.bfloat16, tag="ht")
                nc.scalar.activation(ht_sb[:], ht_ps[:], AF.Relu, scale=1.0 / SCALE)
                # hW2 shard rows nt*RT + j*128 + p
                for j in range(SUB):
                    hw2_ps = pp_small.tile([128, C], DT.float32, tag="small")
                    nc.tensor.matmul(
                        hw2_ps[:],
                        ht_sb[:, j * 128 : (j + 1) * 128],
                        w2_sb[:],
                        start=True,
                        stop=True,
                    )
                    nc.vector.tensor_copy(hw2p_sb[:, nt * SUB + j, :], hw2_ps[:])

            # prefetch pass-2 slabs (sync queue; independent of collectives)
            def load_slab2(nt, ks):
                at2_sb = at_pool.tile([128, KC, RT], DT.float8e4, tag="at")
                nc.sync.dma_start(out=at2_sb[:], in_=adjt8[nt, ks])
                return at2_sb

            # pass-2 consumes nt in reverse so the cached nt=3 tiles are
            # used first (no scratch wait while the hW2 gather lands);
            # prefetch the uncached nt=3 slabs plus the head of nt=2 into
            # the boundary's DMA hole
            nt_order = list(range(NT - 1, -1, -1))
            pre2 = {}
            for ks in range(KS - NCACHE):
                pre2[(NT - 1, ks)] = load_slab2(NT - 1, ks)
            for ks in range(2):
                pre2[(NT - 2, ks)] = load_slab2(NT - 2, ks)

            # ---------------- all-gather hW2 ----------------
            hw2_bounce = dram.tile([R, C], DT.bfloat16)
            nc.sync.dma_start(
                out=hw2_bounce[:, :].rearrange("(s p) n -> p s n", p=128),
                in_=hw2p_sb[:],
            )
            hw2_all = dram.tile([N, C], DT.bfloat16, addr_space="Shared")
            nc.gpsimd.collective_compute(
                "AllGather",
                mybir.AluOpType.bypass,
                replica_groups=[list(range(NCORES))],
                ins=[hw2_bounce.opt()],
                outs=[hw2_all.opt()],
            )
            # contiguous load + 8 PE transposes; fp8 scaled by S2; chunk
            # stride padded to CP=16 bytes for the DoubleRow weight AP
            hw2nat_sb = work.tile([128, N // 128, C], DT.bfloat16, bufs=1)
            nc.sync.dma_start(
                out=hw2nat_sb[:],
                in_=hw2_all[:, :].rearrange("(p r) n -> p r n", p=128),
            )
            hw2f_sb = consts.tile([128, N // 128, CP], DT.float8e4)
            for n in range(C):
                h_ps = pp_tp.tile([128, 128], DT.float32, tag="tp")
                nc.tensor.matmul(
                    h_ps[:], hw2nat_sb[:, :, n], ident_bf[:], start=True, stop=True
                )
                nc.vector.tensor_scalar_mul(hw2f_sb[:, :, n], h_ps[:], S2)

            # ---------------- pass 2: logitsT = (adj_loc @ hW2)^T ------------
            for nt in nt_order:
                lt_ps = pp_acc.tile([C, RT], DT.float32, tag="acc")
                for ks in range(KS):
                    at2_sb = cached.pop((nt, ks), None)
                    if at2_sb is None:
                        at2_sb = pre2.pop((nt, ks), None)
                    if at2_sb is None:
                        at2_sb = load_slab2(nt, ks)
                    for kd in range(KC // 2):
                        k2 = ks * (KC // 2) + kd
                        nc.tensor.matmul(
                            lt_ps[:],
                            hw2f_sb[:, 2 * k2 : 2 * k2 + 2, 0:C],
                            at2_sb[:, 2 * kd : 2 * kd + 2, :],
                            start=(k2 == 0),
                            stop=(k2 == K2 - 1),
                            perf_mode=DR,
                        )
                # epilogue: descale + bias; transpose; log_softmax (no max
                # subtraction -- logits are O(1), exp is safe)
                lt_sb = work.tile([C, RT], DT.float32, tag="lt")
                nc.scalar.activation(
                    lt_sb[:],
                    lt_ps[:],
                    AF.Identity,
                    bias=b2_sb[:],
                    scale=1.0 / (SCALE * S2),
                )
                o_sb = work.tile([128, SUB, C], DT.float32, tag="o")
                for j in range(SUB):
                    ltt_ps = pp_small.tile([128, C], DT.float32, tag="small")
                    nc.tensor.transpose(
                        ltt_ps[:],
                        lt_sb[:, j * 128 : (j + 1) * 128],
                        ident_f32[:C, :C],
                    )
                    e_sb = work.tile([128, C], DT.float32, tag="e")
                    se_sb = work.tile([128, 1], DT.float32, tag="se")
                    nc.scalar.activation(
                        e_sb[:], ltt_ps[:], AF.Exp, accum_out=se_sb[:]
                    )
                    lse_sb = work.tile([128, 1], DT.float32, tag="lse")
                    nc.scalar.activation(lse_sb[:], se_sb[:], AF.Ln)
                    nc.vector.tensor_scalar(
                        o_sb[:, j, :],
                        ltt_ps[:],
                        lse_sb[:],
                        None,
                        op0=mybir.AluOpType.subtract,
                    )
                nc.sync.dma_start(
                    out=out_loc[nt * RT : (nt + 1) * RT, :].rearrange(
                        "(j p) n -> p j n", p=128
                    ),
                    in_=o_sb[:],
                )

    nc.compile()
    return nc


def _get_nc():
    global _cached
    if _cached is None:
        _cached = _build()
    return _cached


last_results = None


def kernel(x, adj, W1, W2, b2):
    global last_results
    x = np.ascontiguousarray(x, dtype=np.float32)
    adj = np.ascontiguousarray(adj, dtype=np.float32)
    W1 = np.ascontiguousarray(W1, dtype=np.float32)
    W2 = np.ascontiguousarray(W2, dtype=np.float32)
    b2 = np.ascontiguousarray(b2, dtype=np.float32).reshape(1, C)

    nc = _get_nc()
    in_maps = [
        {
            "x_loc": x[i * R : (i + 1) * R],
            "adj_loc": adj[i * R : (i + 1) * R],
            "w1_in": W1,
            "w2_in": W2,
            "b2_in": b2,
        }
        for i in range(NCORES)
    ]
    res = run_bass_kernel_spmd(
        nc,
        in_maps,
        core_ids=list(range(NCORES)),
        trace=bool(os.environ.get("GCN_TRACE")),
    )
    last_results = res
    return np.concatenate([res.results[i]["out_loc"] for i in range(NCORES)], axis=0)
